# revision 14
# baseline (speedup 1.0000x reference)
"""Trainium2 Bass kernel for nn_Decoder_24541443129406.

Math: the reference's pdf/pdf_max cancels the normalization, so

    prob[n] = clip( sum_m exp( -0.5 * sum_d (pos[n,d]-mean[m,d])^2 / sigma[m,d] ), 0, 1 )

with pos = [ox, oy, dx, dy], sigma = [sx, sy, 1e-3, 1e-3],
sx = relu(l4)+0.01, sy = relu(l5)+0.01, mean = latents[:, :4].

The exponent is a quadratic form -> a K=8 matmul:
    e[n,m] = f[n] . w[m]
    f[n] = [dx^2+dy^2, 1, ox, oy, dx, dy, ox^2, oy^2]
    w[m] = [c7, c0, c1, c2, c3, c4, c5, c6]
      c1 = mx/sx, c2 = my/sy, c3 = 1000*mdx, c4 = 1000*mdy,
      c5 = -0.5/sx, c6 = -0.5/sy, c7 = -500,
      c0 = -0.5*(mx^2/sx + my^2/sy + 1000*(mdx^2+mdy^2))

fp32 matmuls are 4 cycles/row on the PE and float32r truncates, so the
K=8 fp32 matmul is emulated as one K=24 fp16 matmul with hi/lo split
operands stacked along K: e = h.H + l.H + h.L  (features f = h + l,
weights w = H + L, each half fp16) — ~2^-22 relative accuracy at
1 cycle/row.

Per core (8 cores, data-parallel over rays): N_loc = 8192 rays, M = 512
gaussians. 16 super-tiles of 4 ray-blocks: 4x PE matmul -> PSUM
[128, 2048]; one ACT Exp pass -> fp16 [128, 2048]; per-block fused DVE
tensor_tensor_reduce (fold halves + accumulate) -> per-ray sums; clip;
PE-transpose; contiguous DMA out.
"""

import os
import sys
from contextlib import ExitStack

import numpy as np

for _p in ("/opt/trn_rl_repo", "/root/.axon_site/_ro/trn_rl_repo"):
    if os.path.isdir(_p) and _p not in sys.path:
        sys.path.insert(0, _p)

import concourse.bacc as bacc
import concourse.bass as bass
import concourse.mybir as mybir
import concourse.tile as tile
from concourse import bass_utils
from concourse.masks import make_identity

N_CORES = 8
N = 65536
M = 512
N_LOC = N // N_CORES  # 8192
NCHUNK = 32  # feature-build chunks (32-partition groups: verifier requires
# compute-op SBUF APs to start at partition 0/32/64/96)
CHUNK = N_LOC // NCHUNK  # 256
NBLK = N_LOC // 128  # 64 ray blocks of 128
NSUP = NBLK // 4  # 16 super-tiles of 4 blocks
SIGMA_EPS = 0.01
INV_SDIR = 1000.0  # 1/sigma_dir

F32 = mybir.dt.float32
F16 = mybir.dt.float16
ALU = mybir.AluOpType
ACTF = mybir.ActivationFunctionType

TRACE = False
LAST_PERF = None
_CACHED_NC = None


def build_kernel_body(nc, origins, directions, latents, prob):
    """origins/directions: [N_LOC, 2] f32 DRAM APs; latents [M, 6]; prob [N_LOC, 1]."""
    with tile.TileContext(nc) as tc, ExitStack() as ctx:
        singles = ctx.enter_context(tc.tile_pool(name="singles", bufs=1))
        scratch = ctx.enter_context(tc.tile_pool(name="scratch", bufs=3))

        # ---------------- input loads (parallel DMA queues) ----------------
        # Contiguous loads only; strided extraction happens on-chip where the
        # address generators make it free.
        raw_og = singles.tile([NCHUNK, 2 * CHUNK], F32)
        raw_dr = singles.tile([NCHUNK, 2 * CHUNK], F32)
        og_v = origins.rearrange("(i r) c -> i (r c)", i=NCHUNK)
        dr_v = directions.rearrange("(i r) c -> i (r c)", i=NCHUNK)
        nc.sync.dma_start(out=raw_og[0:16, :], in_=og_v[0:16, :])
        nc.scalar.dma_start(out=raw_og[16:32, :], in_=og_v[16:32, :])
        nc.sync.dma_start(out=raw_dr[0:16, :], in_=dr_v[0:16, :])
        nc.scalar.dma_start(out=raw_dr[16:32, :], in_=dr_v[16:32, :])
        # latents staged contiguously into one partition, rows extracted by
        # strided on-chip copies
        lat_stage = singles.tile([1, M * 6], F32)
        nc.gpsimd.dma_start(out=lat_stage, in_=latents.rearrange("m f -> (m f)")[None, :])
        lat_v = lat_stage.rearrange("o (m f) -> o m f", f=6)

        # ---------------- gaussian weights wg [8, M] ----------------
        # wg rows: 0=c7, 1=c0, 2=c1, 3=c2, 4=c3, 5=c4, 6=c5, 7=c6
        lat_s = singles.tile([2, M], F32)  # lx, ly
        lat_m = singles.tile([2, M], F32)  # mx, my
        lat_d = singles.tile([2, M], F32)  # mdx, mdy
        # strided extracts must start at partition 0 (compute-op rule), so
        # extract each row at partition 0 and DMA it into place
        ext = [singles.tile([1, M], F32, name=f"ext{f}", tag=f"ext{f}") for f in range(6)]
        for f in range(6):
            (nc.vector if f % 2 == 0 else nc.gpsimd).tensor_copy(
                out=ext[f], in_=lat_v[:, :, f]
            )
        nc.sync.dma_start(out=lat_m[0:1, :], in_=ext[0])
        nc.sync.dma_start(out=lat_m[1:2, :], in_=ext[1])
        nc.scalar.dma_start(out=lat_d[0:1, :], in_=ext[2])
        nc.scalar.dma_start(out=lat_d[1:2, :], in_=ext[3])
        nc.gpsimd.dma_start(out=lat_s[0:1, :], in_=ext[4])
        nc.gpsimd.dma_start(out=lat_s[1:2, :], in_=ext[5])

        # s = relu(l)+eps ; r = 1/s   (all ops partition-aligned: rows 0-1)
        nc.vector.tensor_scalar(
            out=lat_s, in0=lat_s, scalar1=0.0, scalar2=SIGMA_EPS,
            op0=ALU.max, op1=ALU.add,
        )
        rec = singles.tile([2, M], F32)
        rec_scr = singles.tile([2, M], F32)
        nc.vector.reciprocal_approx_accurate(out=rec, in_=lat_s, scratch=rec_scr)

        c12 = singles.tile([2, M], F32)  # mx/sx, my/sy
        nc.vector.tensor_mul(out=c12, in0=lat_m, in1=rec)
        c34 = singles.tile([2, M], F32)  # 1000*mdx, 1000*mdy
        nc.vector.tensor_scalar_mul(out=c34, in0=lat_d, scalar1=INV_SDIR)
        c56 = singles.tile([2, M], F32)  # -0.5/sx, -0.5/sy
        nc.vector.tensor_scalar_mul(out=c56, in0=rec, scalar1=-0.5)

        # c0 = -0.5*(mx*c1 + my*c2 + mdx*c3 + mdy*c4)
        p1 = singles.tile([2, M], F32)
        nc.vector.tensor_mul(out=p1, in0=lat_m, in1=c12)
        p2 = singles.tile([2, M], F32)
        nc.vector.tensor_mul(out=p2, in0=lat_d, in1=c34)
        nc.vector.tensor_add(out=p1, in0=p1, in1=p2)
        # cross-partition: move row 1 next to row 0, then aligned add
        p1b = singles.tile([1, M], F32)
        nc.sync.dma_start(out=p1b, in_=p1[1:2, :])
        nc.vector.tensor_add(out=p1[0:1, :], in0=p1[0:1, :], in1=p1b)

        wg = singles.tile([8, M], F32)
        nc.vector.memset(wg[0:1, :], -0.5 * INV_SDIR)  # c7
        nc.vector.tensor_scalar_mul(out=p1[0:1, :], in0=p1[0:1, :], scalar1=-0.5)  # c0
        # assemble (cross-partition moves via DMA)
        nc.sync.dma_start(out=wg[1:2, :], in_=p1[0:1, :])
        nc.sync.dma_start(out=wg[2:4, :], in_=c12)
        nc.sync.dma_start(out=wg[4:6, :], in_=c34)
        nc.sync.dma_start(out=wg[6:8, :], in_=c56)

        # fp16 hi/lo split of the weights; wgs = [H; H; L] stacked along K
        wgh = singles.tile([8, M], F16)
        wgl = singles.tile([8, M], F16)
        nc.vector.tensor_copy(out=wgh, in_=wg)
        nc.vector.tensor_tensor(out=wgl, in0=wg, in1=wgh, op=ALU.subtract)
        wgs = singles.tile([24, M], F16)
        nc.sync.dma_start(out=wgs[0:8, :], in_=wgh)
        nc.scalar.dma_start(out=wgs[8:16, :], in_=wgh)
        nc.sync.dma_start(out=wgs[16:24, :], in_=wgl)

        # ---------------- feature tiles ----------------
        ox, oy = raw_og[:, 0::2], raw_og[:, 1::2]
        dx, dy = raw_dr[:, 0::2], raw_dr[:, 1::2]

        # Two f-major tiles, 32-partition feature groups.
        # featA groups: 0=dx^2+dy^2, 1=ones, 2=ox, 3=oy
        # featB groups: 4=dx, 5=dy, 6=ox^2, 7=oy^2
        featA = singles.tile([128, CHUNK], F32)
        featB = singles.tile([128, CHUNK], F32)
        s1 = singles.tile([NCHUNK, CHUNK], F32)

        nc.vector.tensor_mul(out=featA[0:32, :], in0=dx, in1=dx)
        nc.vector.tensor_mul(out=s1, in0=dy, in1=dy)
        nc.vector.tensor_add(out=featA[0:32, :], in0=featA[0:32, :], in1=s1)
        nc.vector.memset(featA[32:64, :], 1.0)
        nc.gpsimd.tensor_copy(out=featA[64:96, :], in_=ox)
        nc.gpsimd.tensor_copy(out=featA[96:128, :], in_=oy)
        nc.gpsimd.tensor_copy(out=featB[0:32, :], in_=dx)
        nc.gpsimd.tensor_copy(out=featB[32:64, :], in_=dy)
        nc.vector.tensor_mul(out=featB[64:96, :], in0=ox, in1=ox)
        nc.vector.tensor_mul(out=featB[96:128, :], in0=oy, in1=oy)

        # fp16 hi/lo split of the features
        hA = singles.tile([128, CHUNK], F16)
        lA = singles.tile([128, CHUNK], F16)
        hB = singles.tile([128, CHUNK], F16)
        lB = singles.tile([128, CHUNK], F16)
        nc.vector.tensor_copy(out=hA, in_=featA)
        nc.vector.tensor_tensor(out=lA, in0=featA, in1=hA, op=ALU.subtract)
        nc.vector.tensor_copy(out=hB, in_=featB)
        nc.vector.tensor_tensor(out=lB, in0=featB, in1=hB, op=ALU.subtract)

        # permute to featcs [24, N_LOC] fp16: rows 0-7 = h, 8-15 = l,
        # 16-23 = h (the K-stack [h; l; h] pairing with wgs = [H; H; L]:
        # e = h.H + l.H + h.L).  One DMA per feature group: [32, CHUNK]
        # partition-major stream == C-order [1, N_LOC] row.
        featcs = singles.tile([24, N_LOC], F16)
        eng = [nc.sync, nc.scalar, nc.gpsimd, nc.sync]
        for f in range(4):
            hsrcA = hA[32 * f : 32 * (f + 1), :]
            hsrcB = hB[32 * f : 32 * (f + 1), :]
            eng[f].dma_start(out=featcs[f : f + 1, :], in_=hsrcA)
            eng[f].dma_start(out=featcs[4 + f : 5 + f, :], in_=hsrcB)
            eng[f].dma_start(out=featcs[16 + f : 17 + f, :], in_=hsrcA)
            eng[f].dma_start(out=featcs[20 + f : 21 + f, :], in_=hsrcB)
            eng[f].dma_start(
                out=featcs[8 + f : 9 + f, :], in_=lA[32 * f : 32 * (f + 1), :]
            )
            eng[f].dma_start(
                out=featcs[12 + f : 13 + f, :], in_=lB[32 * f : 32 * (f + 1), :]
            )

        # identity for the output transpose (one-time, overlaps setup)
        ident = singles.tile([128, 128], F32)
        make_identity(nc, ident)

        # ---------------- main loop ----------------
        res = singles.tile([128, NBLK], F32)  # res[p, b] = sum_m exp(e), ray 128b+p
        with tc.tile_pool(name="psum", bufs=2, space="PSUM") as psum:
            for s in range(NSUP):
                ps = psum.tile([128, 4 * M], F32, tag="ps")
                for j in range(4):
                    b = 4 * s + j
                    nc.tensor.matmul(
                        out=ps[:, M * j : M * (j + 1)],
                        lhsT=featcs[:, 128 * b : 128 * (b + 1)],
                        rhs=wgs,
                        start=True,
                        stop=True,
                    )
                ex = scratch.tile([128, 4 * M], F16, tag="ex")
                nc.scalar.activation(out=ex, in_=ps, func=ACTF.Exp)
                for j in range(4):
                    b = 4 * s + j
                    nc.vector.tensor_reduce(
                        out=res[:, b : b + 1],
                        in_=ex[:, M * j : M * (j + 1)],
                        axis=mybir.AxisListType.X,
                        op=ALU.add,
                    )

        # clip to [0, 1]
        nc.vector.tensor_scalar(
            out=res, in0=res, scalar1=0.0, scalar2=1.0, op0=ALU.max, op1=ALU.min
        )

        # transpose [128, NBLK] -> [NBLK, 128] so DRAM writes are contiguous
        with tc.tile_pool(name="psumt", bufs=1, space="PSUM") as psumt:
            pst = psumt.tile([NBLK, 128], F32)
            nc.tensor.transpose(out=pst, in_=res[:, :], identity=ident[:, :])
            rest = singles.tile([NBLK, 128], F32)
            nc.vector.tensor_copy(out=rest, in_=pst)
            nc.sync.dma_start(
                out=prob.rearrange("(b p) o -> b (p o)", b=NBLK), in_=rest
            )


def build_nc():
    nc = bacc.Bacc("TRN2", target_bir_lowering=False, debug=False)
    origins = nc.dram_tensor("origins", [N_LOC, 2], F32, kind="ExternalInput").ap()
    directions = nc.dram_tensor("directions", [N_LOC, 2], F32, kind="ExternalInput").ap()
    latents = nc.dram_tensor("latents", [M, 6], F32, kind="ExternalInput").ap()
    prob = nc.dram_tensor("prob", [N_LOC, 1], F32, kind="ExternalOutput").ap()
    build_kernel_body(nc, origins, directions, latents, prob)
    nc.compile()
    return nc


def kernel(origins: np.ndarray, directions: np.ndarray, latents: np.ndarray) -> np.ndarray:
    global _CACHED_NC, LAST_PERF
    assert origins.shape == (N, 2) and directions.shape == (N, 2)
    assert latents.shape == (M, 6)
    origins = np.ascontiguousarray(origins, dtype=np.float32)
    directions = np.ascontiguousarray(directions, dtype=np.float32)
    latents = np.ascontiguousarray(latents, dtype=np.float32)

    if _CACHED_NC is None:
        _CACHED_NC = build_nc()
    nc = _CACHED_NC

    in_maps = []
    for c in range(N_CORES):
        sl = slice(c * N_LOC, (c + 1) * N_LOC)
        in_maps.append(
            {
                "origins": origins[sl],
                "directions": directions[sl],
                "latents": latents,
            }
        )

    results = bass_utils.run_bass_kernel_spmd(
        nc,
        in_maps,
        core_ids=list(range(N_CORES)),
        trace=TRACE,
    )
    LAST_PERF = results
    out = np.concatenate([results.results[c]["prob"] for c in range(N_CORES)], axis=0)
    return out.astype(np.float32)


if __name__ == "__main__":
    rng = np.random.default_rng(0)
    o = rng.standard_normal((N, 2), dtype=np.float32)
    d = rng.standard_normal((N, 2), dtype=np.float32)
    l = rng.standard_normal((M, 6), dtype=np.float32)
    p = kernel(o, d, l)
    print(p.shape, p.dtype, p.min(), p.max())


# revision 17
# speedup vs baseline: 3156.2249x; 3156.2249x over previous
"""Trainium2 Bass kernel for nn_Decoder_24541443129406.

Math: the reference's pdf/pdf_max cancels the normalization, so

    prob[n] = clip( sum_m exp( -0.5 * sum_d (pos[n,d]-mean[m,d])^2 / sigma[m,d] ), 0, 1 )

with pos = [ox, oy, dx, dy], sigma = [sx, sy, 1e-3, 1e-3],
sx = relu(l4)+0.01, sy = relu(l5)+0.01, mean = latents[:, :4].

The exponent is a quadratic form -> a K=8 matmul:
    e[n,m] = f[n] . w[m]
    f[n] = [dx^2+dy^2, 1, ox, oy, dx, dy, ox^2, oy^2]
    w[m] = [c7, c0, c1, c2, c3, c4, c5, c6]
      c1 = mx/sx, c2 = my/sy, c3 = 1000*mdx, c4 = 1000*mdy,
      c5 = -0.5/sx, c6 = -0.5/sy, c7 = -500,
      c0 = -0.5*(mx^2/sx + my^2/sy + 1000*(mdx^2+mdy^2))

fp32 matmuls are 4 cycles/row on the PE and float32r truncates, so the
K=8 fp32 matmul is emulated as one K=24 fp16 matmul with hi/lo split
operands stacked along K: e = h.H + l.H + h.L  (features f = h + l,
weights w = H + L, each half fp16) — ~2^-22 relative accuracy at
1 cycle/row.

Per core (8 cores, data-parallel over rays): N_loc = 8192 rays, M = 512
gaussians. 16 super-tiles of 4 ray-blocks: 4x PE matmul -> PSUM
[128, 2048]; one ACT Exp pass -> fp16 [128, 2048]; per-block fused DVE
tensor_tensor_reduce (fold halves + accumulate) -> per-ray sums; clip;
PE-transpose; contiguous DMA out.
"""

import os
import sys
from contextlib import ExitStack

import numpy as np

for _p in ("/opt/trn_rl_repo", "/root/.axon_site/_ro/trn_rl_repo"):
    if os.path.isdir(_p) and _p not in sys.path:
        sys.path.insert(0, _p)

import concourse.bacc as bacc
import concourse.bass as bass
import concourse.mybir as mybir
import concourse.tile as tile
from concourse import bass_utils
from concourse.masks import make_identity

N_CORES = 8
N = 65536
M = 512
N_LOC = N // N_CORES  # 8192
NCHUNK = 32  # feature-build chunks (32-partition groups: verifier requires
# compute-op SBUF APs to start at partition 0/32/64/96)
CHUNK = N_LOC // NCHUNK  # 256
NBLK = N_LOC // 128  # 64 ray blocks of 128
NSUP = NBLK // 4  # 16 super-tiles of 4 blocks
SIGMA_EPS = 0.01
INV_SDIR = 1000.0  # 1/sigma_dir

F32 = mybir.dt.float32
F16 = mybir.dt.float16
ALU = mybir.AluOpType
ACTF = mybir.ActivationFunctionType

TRACE = False
LAST_PERF = None
_CACHED_NC = None


def build_kernel_body(nc, origins, directions, latents, prob):
    """origins/directions: [N_LOC, 2] f32 DRAM APs; latents [M, 6]; prob [N_LOC, 1]."""
    with tile.TileContext(nc) as tc, ExitStack() as ctx:
        singles = ctx.enter_context(tc.tile_pool(name="singles", bufs=1))
        scratch = ctx.enter_context(tc.tile_pool(name="scratch", bufs=3))

        # ---------------- input loads (parallel DMA queues) ----------------
        # Contiguous loads only; strided extraction happens on-chip where the
        # address generators make it free.
        raw_og = singles.tile([NCHUNK, 2 * CHUNK], F32)
        raw_dr = singles.tile([NCHUNK, 2 * CHUNK], F32)
        og_v = origins.rearrange("(i r) c -> i (r c)", i=NCHUNK)
        dr_v = directions.rearrange("(i r) c -> i (r c)", i=NCHUNK)
        nc.sync.dma_start(out=raw_og[0:16, :], in_=og_v[0:16, :])
        nc.scalar.dma_start(out=raw_og[16:32, :], in_=og_v[16:32, :])
        nc.sync.dma_start(out=raw_dr[0:16, :], in_=dr_v[0:16, :])
        nc.scalar.dma_start(out=raw_dr[16:32, :], in_=dr_v[16:32, :])
        # latents staged contiguously into one partition, rows extracted by
        # strided on-chip copies
        lat_stage = singles.tile([1, M * 6], F32)
        nc.gpsimd.dma_start(out=lat_stage, in_=latents.rearrange("m f -> (m f)")[None, :])
        lat_v = lat_stage.rearrange("o (m f) -> o m f", f=6)

        # ---------------- gaussian weights wg [8, M] ----------------
        # wg rows: 0=c7, 1=c0, 2=c1, 3=c2, 4=c3, 5=c4, 6=c5, 7=c6
        lat_s = singles.tile([2, M], F32)  # lx, ly
        lat_m = singles.tile([2, M], F32)  # mx, my
        lat_d = singles.tile([2, M], F32)  # mdx, mdy
        # strided extracts must start at partition 0 (compute-op rule), so
        # extract each row at partition 0 and DMA it into place
        ext = [singles.tile([1, M], F32, name=f"ext{f}", tag=f"ext{f}") for f in range(6)]
        for f in range(6):
            (nc.vector if f % 2 == 0 else nc.gpsimd).tensor_copy(
                out=ext[f], in_=lat_v[:, :, f]
            )
        nc.sync.dma_start(out=lat_m[0:1, :], in_=ext[0])
        nc.sync.dma_start(out=lat_m[1:2, :], in_=ext[1])
        nc.scalar.dma_start(out=lat_d[0:1, :], in_=ext[2])
        nc.scalar.dma_start(out=lat_d[1:2, :], in_=ext[3])
        nc.gpsimd.dma_start(out=lat_s[0:1, :], in_=ext[4])
        nc.gpsimd.dma_start(out=lat_s[1:2, :], in_=ext[5])

        # s = relu(l)+eps ; r = 1/s   (all ops partition-aligned: rows 0-1)
        nc.vector.tensor_scalar(
            out=lat_s, in0=lat_s, scalar1=0.0, scalar2=SIGMA_EPS,
            op0=ALU.max, op1=ALU.add,
        )
        rec = singles.tile([2, M], F32)
        rec_scr = singles.tile([2, M], F32)
        nc.vector.reciprocal_approx_accurate(out=rec, in_=lat_s, scratch=rec_scr)

        c12 = singles.tile([2, M], F32)  # mx/sx, my/sy
        nc.vector.tensor_mul(out=c12, in0=lat_m, in1=rec)
        c34 = singles.tile([2, M], F32)  # 1000*mdx, 1000*mdy
        nc.vector.tensor_scalar_mul(out=c34, in0=lat_d, scalar1=INV_SDIR)
        c56 = singles.tile([2, M], F32)  # -0.5/sx, -0.5/sy
        nc.vector.tensor_scalar_mul(out=c56, in0=rec, scalar1=-0.5)

        # c0 = -0.5*(mx*c1 + my*c2 + mdx*c3 + mdy*c4)
        p1 = singles.tile([2, M], F32)
        nc.vector.tensor_mul(out=p1, in0=lat_m, in1=c12)
        p2 = singles.tile([2, M], F32)
        nc.vector.tensor_mul(out=p2, in0=lat_d, in1=c34)
        nc.vector.tensor_add(out=p1, in0=p1, in1=p2)
        # cross-partition: move row 1 next to row 0, then aligned add
        p1b = singles.tile([1, M], F32)
        nc.sync.dma_start(out=p1b, in_=p1[1:2, :])
        nc.vector.tensor_add(out=p1[0:1, :], in0=p1[0:1, :], in1=p1b)

        wg = singles.tile([8, M], F32)
        nc.vector.memset(wg[0:1, :], -0.5 * INV_SDIR)  # c7
        nc.vector.tensor_scalar_mul(out=p1[0:1, :], in0=p1[0:1, :], scalar1=-0.5)  # c0
        # assemble (cross-partition moves via DMA)
        nc.sync.dma_start(out=wg[1:2, :], in_=p1[0:1, :])
        nc.sync.dma_start(out=wg[2:4, :], in_=c12)
        nc.sync.dma_start(out=wg[4:6, :], in_=c34)
        nc.sync.dma_start(out=wg[6:8, :], in_=c56)

        # fp16 hi/lo split of the weights; wgs = [H; H; L] stacked along K
        wgh = singles.tile([8, M], F16)
        wgl = singles.tile([8, M], F16)
        nc.vector.tensor_copy(out=wgh, in_=wg)
        nc.vector.tensor_tensor(out=wgl, in0=wg, in1=wgh, op=ALU.subtract)
        wgs = singles.tile([24, M], F16)
        nc.sync.dma_start(out=wgs[0:8, :], in_=wgh)
        nc.scalar.dma_start(out=wgs[8:16, :], in_=wgh)
        nc.sync.dma_start(out=wgs[16:24, :], in_=wgl)

        # ---------------- feature tiles ----------------
        ox, oy = raw_og[:, 0::2], raw_og[:, 1::2]
        dx, dy = raw_dr[:, 0::2], raw_dr[:, 1::2]

        # Two f-major tiles, 32-partition feature groups.
        # featA groups: 0=dx^2+dy^2, 1=ones, 2=ox, 3=oy
        # featB groups: 4=dx, 5=dy, 6=ox^2, 7=oy^2
        featA = singles.tile([128, CHUNK], F32)
        featB = singles.tile([128, CHUNK], F32)
        s1 = singles.tile([NCHUNK, CHUNK], F32)

        nc.vector.tensor_mul(out=featA[0:32, :], in0=dx, in1=dx)
        nc.vector.tensor_mul(out=s1, in0=dy, in1=dy)
        nc.vector.tensor_add(out=featA[0:32, :], in0=featA[0:32, :], in1=s1)
        nc.vector.memset(featA[32:64, :], 1.0)
        nc.gpsimd.tensor_copy(out=featA[64:96, :], in_=ox)
        nc.gpsimd.tensor_copy(out=featA[96:128, :], in_=oy)
        nc.gpsimd.tensor_copy(out=featB[0:32, :], in_=dx)
        nc.gpsimd.tensor_copy(out=featB[32:64, :], in_=dy)
        nc.vector.tensor_mul(out=featB[64:96, :], in0=ox, in1=ox)
        nc.vector.tensor_mul(out=featB[96:128, :], in0=oy, in1=oy)

        # fp16 hi/lo split of the features
        hA = singles.tile([128, CHUNK], F16)
        lA = singles.tile([128, CHUNK], F16)
        hB = singles.tile([128, CHUNK], F16)
        lB = singles.tile([128, CHUNK], F16)
        nc.vector.tensor_copy(out=hA, in_=featA)
        nc.vector.tensor_tensor(out=lA, in0=featA, in1=hA, op=ALU.subtract)
        nc.vector.tensor_copy(out=hB, in_=featB)
        nc.vector.tensor_tensor(out=lB, in0=featB, in1=hB, op=ALU.subtract)

        # permute to featcs [24, N_LOC] fp16: rows 0-7 = h, 8-15 = l,
        # 16-23 = h (the K-stack [h; l; h] pairing with wgs = [H; H; L]:
        # e = h.H + l.H + h.L).  One DMA per feature group: [32, CHUNK]
        # partition-major stream == C-order [1, N_LOC] row.
        featcs = singles.tile([24, N_LOC], F16)
        eng = [nc.sync, nc.scalar, nc.gpsimd, nc.sync]
        for f in range(4):
            hsrcA = hA[32 * f : 32 * (f + 1), :]
            hsrcB = hB[32 * f : 32 * (f + 1), :]
            eng[f].dma_start(out=featcs[f : f + 1, :], in_=hsrcA)
            eng[f].dma_start(out=featcs[4 + f : 5 + f, :], in_=hsrcB)
            eng[f].dma_start(out=featcs[16 + f : 17 + f, :], in_=hsrcA)
            eng[f].dma_start(out=featcs[20 + f : 21 + f, :], in_=hsrcB)
            eng[f].dma_start(
                out=featcs[8 + f : 9 + f, :], in_=lA[32 * f : 32 * (f + 1), :]
            )
            eng[f].dma_start(
                out=featcs[12 + f : 13 + f, :], in_=lB[32 * f : 32 * (f + 1), :]
            )

        # identity for the output transpose (one-time, overlaps setup)
        ident = singles.tile([128, 128], F32)
        make_identity(nc, ident)

        # ---------------- main loop ----------------
        res = singles.tile([128, NBLK], F32)  # res[p, b] = sum_m exp(e), ray 128b+p
        with tc.tile_pool(name="psum", bufs=2, space="PSUM") as psum:
            for s in range(NSUP):
                ps = psum.tile([128, 4 * M], F32, tag="ps")
                for j in range(4):
                    b = 4 * s + j
                    nc.tensor.matmul(
                        out=ps[:, M * j : M * (j + 1)],
                        lhsT=featcs[:, 128 * b : 128 * (b + 1)],
                        rhs=wgs,
                        start=True,
                        stop=True,
                    )
                ex = scratch.tile([128, 4 * M], F32, tag="ex")
                nc.scalar.activation(out=ex, in_=ps, func=ACTF.Exp)
                for j in range(4):
                    b = 4 * s + j
                    sc = scratch.tile([128, M], F32, tag="sc")
                    nc.vector.tensor_scalar(
                        out=sc,
                        in0=ex[:, M * j : M * (j + 1)],
                        scalar1=0.0,
                        scalar2=0.0,
                        op0=ALU.add,
                        op1=ALU.add,
                        accum_out=res[:, b : b + 1],
                    )

        # clip to [0, 1]
        nc.vector.tensor_scalar(
            out=res, in0=res, scalar1=0.0, scalar2=1.0, op0=ALU.max, op1=ALU.min
        )

        # transpose [128, NBLK] -> [NBLK, 128] so DRAM writes are contiguous
        with tc.tile_pool(name="psumt", bufs=1, space="PSUM") as psumt:
            pst = psumt.tile([NBLK, 128], F32)
            nc.tensor.transpose(out=pst, in_=res[:, :], identity=ident[:, :])
            rest = singles.tile([NBLK, 128], F32)
            nc.vector.tensor_copy(out=rest, in_=pst)
            nc.sync.dma_start(
                out=prob.rearrange("(b p) o -> b (p o)", b=NBLK), in_=rest
            )


def build_nc():
    nc = bacc.Bacc("TRN2", target_bir_lowering=False, debug=False)
    origins = nc.dram_tensor("origins", [N_LOC, 2], F32, kind="ExternalInput").ap()
    directions = nc.dram_tensor("directions", [N_LOC, 2], F32, kind="ExternalInput").ap()
    latents = nc.dram_tensor("latents", [M, 6], F32, kind="ExternalInput").ap()
    prob = nc.dram_tensor("prob", [N_LOC, 1], F32, kind="ExternalOutput").ap()
    build_kernel_body(nc, origins, directions, latents, prob)
    nc.compile()
    return nc


def kernel(origins: np.ndarray, directions: np.ndarray, latents: np.ndarray) -> np.ndarray:
    global _CACHED_NC, LAST_PERF
    assert origins.shape == (N, 2) and directions.shape == (N, 2)
    assert latents.shape == (M, 6)
    origins = np.ascontiguousarray(origins, dtype=np.float32)
    directions = np.ascontiguousarray(directions, dtype=np.float32)
    latents = np.ascontiguousarray(latents, dtype=np.float32)

    if _CACHED_NC is None:
        _CACHED_NC = build_nc()
    nc = _CACHED_NC

    in_maps = []
    for c in range(N_CORES):
        sl = slice(c * N_LOC, (c + 1) * N_LOC)
        in_maps.append(
            {
                "origins": origins[sl],
                "directions": directions[sl],
                "latents": latents,
            }
        )

    results = bass_utils.run_bass_kernel_spmd(
        nc,
        in_maps,
        core_ids=list(range(N_CORES)),
        trace=TRACE,
    )
    LAST_PERF = results
    out = np.concatenate([results.results[c]["prob"] for c in range(N_CORES)], axis=0)
    return out.astype(np.float32)


if __name__ == "__main__":
    rng = np.random.default_rng(0)
    o = rng.standard_normal((N, 2), dtype=np.float32)
    d = rng.standard_normal((N, 2), dtype=np.float32)
    l = rng.standard_normal((M, 6), dtype=np.float32)
    p = kernel(o, d, l)
    print(p.shape, p.dtype, p.min(), p.max())


# revision 21
# speedup vs baseline: 3512.8684x; 1.1130x over previous
"""Trainium2 Bass kernel for nn_Decoder_24541443129406.

Math: the reference's pdf/pdf_max cancels the normalization, so

    prob[n] = clip( sum_m exp( -0.5 * sum_d (pos[n,d]-mean[m,d])^2 / sigma[m,d] ), 0, 1 )

with pos = [ox, oy, dx, dy], sigma = [sx, sy, 1e-3, 1e-3],
sx = relu(l4)+0.01, sy = relu(l5)+0.01, mean = latents[:, :4].

The exponent is a quadratic form -> a K=8 matmul:
    e[n,m] = f[n] . w[m]
    f[n] = [dx^2+dy^2, 1, ox, oy, dx, dy, ox^2, oy^2]
    w[m] = [c7, c0, c1, c2, c3, c4, c5, c6]
      c1 = mx/sx, c2 = my/sy, c3 = 1000*mdx, c4 = 1000*mdy,
      c5 = -0.5/sx, c6 = -0.5/sy, c7 = -500,
      c0 = -0.5*(mx^2/sx + my^2/sy + 1000*(mdx^2+mdy^2))

fp32 matmuls are 4 cycles/row on the PE and float32r truncates, so the
K=8 fp32 matmul is emulated as one K=24 fp16 matmul with hi/lo split
operands stacked along K: e = h.H + l.H + h.L  (features f = h + l,
weights w = H + L, each half fp16; fp16 x fp16 products are exact in
fp32) — ~2^-22 relative accuracy at 1 cycle/row.

Per core (8 cores, data-parallel over rays): N_loc = 8192 rays, M = 512
gaussians. 16 super-tiles of 4 ray-blocks: 4x PE matmul -> PSUM
[128, 2048]; one ACT Exp pass -> fp16 [128, 2048] SBUF; per-block DVE
tensor_scalar with accum_out -> per-ray sums; clip; PE-transpose;
contiguous DMA out.
"""

import os
import sys
from contextlib import ExitStack

import numpy as np

for _p in ("/opt/trn_rl_repo", "/root/.axon_site/_ro/trn_rl_repo"):
    if os.path.isdir(_p) and _p not in sys.path:
        sys.path.insert(0, _p)

import concourse.bacc as bacc
import concourse.bass as bass
import concourse.mybir as mybir
import concourse.tile as tile
from concourse import bass_utils
from concourse.masks import make_identity

N_CORES = 8
N = 65536
M = 512
N_LOC = N // N_CORES  # 8192
NCHUNK = 32  # feature-build chunks (32-partition groups: verifier requires
# compute-op SBUF APs to start at partition 0/32/64/96)
CHUNK = N_LOC // NCHUNK  # 256
NBLK = N_LOC // 128  # 64 ray blocks of 128
NSUP = NBLK // 4  # 16 super-tiles of 4 blocks
SIGMA_EPS = 0.01
INV_SDIR = 1000.0  # 1/sigma_dir

F32 = mybir.dt.float32
F16 = mybir.dt.float16
ALU = mybir.AluOpType
ACTF = mybir.ActivationFunctionType

TRACE = False
LAST_PERF = None
_CACHED_NC = None


def build_kernel_body(nc, origins, directions, latents, prob):
    """origins/directions: [N_LOC, 2] f32 DRAM APs; latents [M, 6]; prob [N_LOC, 1]."""
    with tile.TileContext(nc) as tc, ExitStack() as ctx:
        singles = ctx.enter_context(tc.tile_pool(name="singles", bufs=1))
        scratch = ctx.enter_context(tc.tile_pool(name="scratch", bufs=3))

        # ---------------- input loads (parallel DMA queues) ----------------
        # Contiguous loads only; strided extraction happens on-chip where the
        # address generators make it free.
        raw_og = singles.tile([NCHUNK, 2 * CHUNK], F32)
        raw_dr = singles.tile([NCHUNK, 2 * CHUNK], F32)
        og_v = origins.rearrange("(i r) c -> i (r c)", i=NCHUNK)
        dr_v = directions.rearrange("(i r) c -> i (r c)", i=NCHUNK)
        nc.sync.dma_start(out=raw_dr[0:16, :], in_=dr_v[0:16, :])
        nc.scalar.dma_start(out=raw_dr[16:32, :], in_=dr_v[16:32, :])
        nc.gpsimd.dma_start(out=raw_og[0:16, :], in_=og_v[0:16, :])
        nc.sync.dma_start(out=raw_og[16:32, :], in_=og_v[16:32, :])
        # latents contiguous: partition p = 16 latents (16*6 = 96 floats)
        lat32 = singles.tile([32, 96], F32)
        nc.scalar.dma_start(
            out=lat32, in_=latents.rearrange("(p j) f -> p (j f)", p=32)
        )

        # ---------------- gaussian weights ----------------
        # 32-lane strided extracts [32, 16], then tiny DMAs into paired rows
        # (value (p, j) = feature f of latent m = 16p + j; partition-major
        # stream == m-order).
        exf = [
            singles.tile([32, 16], F32, name=f"exf{f}", tag=f"exf{f}")
            for f in range(6)
        ]
        for f in range(6):
            nc.vector.tensor_copy(out=exf[f], in_=lat32[:, f::6])
        lat_s = singles.tile([2, M], F32)  # lx, ly
        lat_m = singles.tile([2, M], F32)  # mx, my
        lat_d = singles.tile([2, M], F32)  # mdx, mdy
        nc.gpsimd.dma_start(out=lat_s[0:1, :], in_=exf[4])
        nc.gpsimd.dma_start(out=lat_s[1:2, :], in_=exf[5])
        nc.sync.dma_start(out=lat_m[0:1, :], in_=exf[0])
        nc.sync.dma_start(out=lat_m[1:2, :], in_=exf[1])
        nc.scalar.dma_start(out=lat_d[0:1, :], in_=exf[2])
        nc.scalar.dma_start(out=lat_d[1:2, :], in_=exf[3])

        # s = relu(l)+eps ; rec = 1/s  (critical chain, keep short)
        nc.vector.tensor_scalar(
            out=lat_s, in0=lat_s, scalar1=0.0, scalar2=SIGMA_EPS,
            op0=ALU.max, op1=ALU.add,
        )
        rec = singles.tile([2, M], F32)
        rec_scr = singles.tile([2, M], F32)
        nc.vector.reciprocal_approx_accurate(out=rec, in_=lat_s, scratch=rec_scr)

        # recip-independent pieces (overlap with the recip chain)
        msq_s = singles.tile([2, M], F32)  # -0.5 * m^2
        nc.gpsimd.tensor_mul(out=msq_s, in0=lat_m, in1=lat_m)
        nc.scalar.mul(out=msq_s, in_=msq_s, mul=-0.5)
        p2 = singles.tile([2, M], F32)  # -0.5 * 1000 * md^2
        nc.gpsimd.tensor_mul(out=p2, in0=lat_d, in1=lat_d)
        nc.scalar.mul(out=p2, in_=p2, mul=-0.5 * INV_SDIR)
        c34 = singles.tile([2, M], F32)  # 1000*mdx, 1000*mdy
        nc.scalar.mul(out=c34, in_=lat_d, mul=INV_SDIR)

        # recip-dependent pieces
        c12 = singles.tile([2, M], F32)  # mx/sx, my/sy
        nc.vector.tensor_mul(out=c12, in0=lat_m, in1=rec)
        c56 = singles.tile([2, M], F32)  # -0.5/sx, -0.5/sy
        nc.scalar.mul(out=c56, in_=rec, mul=-0.5)
        q = singles.tile([2, M], F32)  # -0.5*(m^2/s) + -500*md^2 per axis
        nc.vector.tensor_mul(out=q, in0=msq_s, in1=rec)
        nc.vector.tensor_add(out=q, in0=q, in1=p2)
        # cross-partition fold: move row 1 next to row 0, aligned add -> c0
        qb = singles.tile([1, M], F32)
        nc.sync.dma_start(out=qb, in_=q[1:2, :])
        c0 = singles.tile([1, M], F32)
        nc.vector.tensor_add(out=c0, in0=q[0:1, :], in1=qb)

        # fp16 hi/lo split, assembled directly into the stacked weight
        # tile wgs [24, M]: rows 0-7 = H, 8-15 = H, 16-23 = L
        # (row order within each group: 0=c7, 1=c0, 2-3=c12, 4-5=c34, 6-7=c56)
        wgs = singles.tile([24, M], F16)
        c7h = singles.tile([1, M], F16)
        c7l = singles.tile([1, M], F16)
        nc.vector.memset(c7h, -0.5 * INV_SDIR)
        nc.vector.memset(c7l, 0.0)
        nc.sync.dma_start(out=wgs[0:1, :], in_=c7h)
        nc.scalar.dma_start(out=wgs[8:9, :], in_=c7h)
        nc.gpsimd.dma_start(out=wgs[16:17, :], in_=c7l)

        def split16(piece, rows, dve, name):
            # hi/lo fp16 split of `piece` ([r, M] f32) + 3 DMAs into wgs,
            # `rows` = row offset within each 8-row group
            r = piece.shape[0]
            h = singles.tile([r, M], F16, name=f"{name}_h", tag=f"{name}_h")
            lo = singles.tile([r, M], F16, name=f"{name}_l", tag=f"{name}_l")
            dve.tensor_copy(out=h, in_=piece)
            dve.tensor_tensor(out=lo, in0=piece, in1=h, op=ALU.subtract)
            nc.sync.dma_start(out=wgs[rows : rows + r, :], in_=h)
            nc.scalar.dma_start(out=wgs[8 + rows : 8 + rows + r, :], in_=h)
            nc.gpsimd.dma_start(out=wgs[16 + rows : 16 + rows + r, :], in_=lo)

        split16(c0, 1, nc.vector, "c0")
        split16(c12, 2, nc.vector, "c12")
        split16(c34, 4, nc.gpsimd, "c34")
        split16(c56, 6, nc.vector, "c56")

        # ---------------- feature tiles ----------------
        ox, oy = raw_og[:, 0::2], raw_og[:, 1::2]
        dx, dy = raw_dr[:, 0::2], raw_dr[:, 1::2]

        # Two f-major tiles, 32-partition feature groups.
        # featA groups: 0=dx^2+dy^2, 1=ones, 2=ox, 3=oy
        # featB groups: 4=dx, 5=dy, 6=ox^2, 7=oy^2
        featA = singles.tile([128, CHUNK], F32)
        featB = singles.tile([128, CHUNK], F32)
        s1 = singles.tile([NCHUNK, CHUNK], F32)

        nc.vector.tensor_mul(out=featA[0:32, :], in0=dx, in1=dx)
        nc.vector.tensor_mul(out=s1, in0=dy, in1=dy)
        nc.vector.tensor_add(out=featA[0:32, :], in0=featA[0:32, :], in1=s1)
        nc.vector.memset(featA[32:64, :], 1.0)
        nc.scalar.copy(out=featA[64:96, :], in_=ox)
        nc.scalar.copy(out=featA[96:128, :], in_=oy)
        nc.gpsimd.tensor_copy(out=featB[0:32, :], in_=dx)
        nc.gpsimd.tensor_copy(out=featB[32:64, :], in_=dy)
        nc.vector.tensor_mul(out=featB[64:96, :], in0=ox, in1=ox)
        nc.vector.tensor_mul(out=featB[96:128, :], in0=oy, in1=oy)

        # fp16 hi/lo split of the features
        hA = singles.tile([128, CHUNK], F16)
        lA = singles.tile([128, CHUNK], F16)
        hB = singles.tile([128, CHUNK], F16)
        lB = singles.tile([128, CHUNK], F16)
        nc.vector.tensor_copy(out=hA, in_=featA)
        nc.vector.tensor_tensor(out=lA, in0=featA, in1=hA, op=ALU.subtract)
        nc.vector.tensor_copy(out=hB, in_=featB)
        nc.gpsimd.tensor_tensor(out=lB, in0=featB, in1=hB, op=ALU.subtract)

        # permute to featcs [24, N_LOC] fp16: rows 0-7 = h, 8-15 = l,
        # 16-23 = h (K-stack [h; l; h] paired with wgs = [H; H; L]:
        # e = h.H + l.H + h.L).  One DMA per feature group: [32, CHUNK]
        # partition-major stream == C-order [1, N_LOC] row.
        featcs = singles.tile([24, N_LOC], F16)
        eng = [nc.sync, nc.scalar, nc.gpsimd, nc.sync]
        for f in range(4):
            hsrcA = hA[32 * f : 32 * (f + 1), :]
            hsrcB = hB[32 * f : 32 * (f + 1), :]
            eng[f].dma_start(out=featcs[f : f + 1, :], in_=hsrcA)
            eng[f].dma_start(out=featcs[4 + f : 5 + f, :], in_=hsrcB)
            eng[f].dma_start(out=featcs[16 + f : 17 + f, :], in_=hsrcA)
            eng[f].dma_start(out=featcs[20 + f : 21 + f, :], in_=hsrcB)
            eng[(f + 1) % 3].dma_start(
                out=featcs[8 + f : 9 + f, :], in_=lA[32 * f : 32 * (f + 1), :]
            )
            eng[(f + 2) % 3].dma_start(
                out=featcs[12 + f : 13 + f, :], in_=lB[32 * f : 32 * (f + 1), :]
            )

        # identity for the output transpose (one-time, overlaps setup)
        ident = singles.tile([128, 128], F32)
        make_identity(nc, ident)

        # ---------------- main loop ----------------
        res = singles.tile([128, NBLK], F32)  # res[p, b] = sum_m exp(e), ray 128b+p
        with tc.tile_pool(name="psum", bufs=2, space="PSUM") as psum:
            for s in range(NSUP):
                ps = psum.tile([128, 4 * M], F32, tag="ps")
                for j in range(4):
                    b = 4 * s + j
                    nc.tensor.matmul(
                        out=ps[:, M * j : M * (j + 1)],
                        lhsT=featcs[:, 128 * b : 128 * (b + 1)],
                        rhs=wgs,
                        start=True,
                        stop=True,
                    )
                ex = scratch.tile([128, 4 * M], F16, tag="ex")
                nc.scalar.activation(out=ex, in_=ps, func=ACTF.Exp)
                for j in range(4):
                    b = 4 * s + j
                    sc = scratch.tile([128, M], F16, tag="sc")
                    nc.vector.tensor_scalar(
                        out=sc,
                        in0=ex[:, M * j : M * (j + 1)],
                        scalar1=0.0,
                        scalar2=0.0,
                        op0=ALU.add,
                        op1=ALU.add,
                        accum_out=res[:, b : b + 1],
                    )

        # clip to [0, 1]
        nc.vector.tensor_scalar(
            out=res, in0=res, scalar1=0.0, scalar2=1.0, op0=ALU.max, op1=ALU.min
        )

        # transpose [128, NBLK] -> [NBLK, 128] so DRAM writes are contiguous
        with tc.tile_pool(name="psumt", bufs=1, space="PSUM") as psumt:
            pst = psumt.tile([NBLK, 128], F32)
            nc.tensor.transpose(out=pst, in_=res[:, :], identity=ident[:, :])
            rest = singles.tile([NBLK, 128], F32)
            nc.vector.tensor_copy(out=rest, in_=pst)
            nc.sync.dma_start(
                out=prob.rearrange("(b p) o -> b (p o)", b=NBLK), in_=rest
            )


def build_nc():
    nc = bacc.Bacc("TRN2", target_bir_lowering=False, debug=False)
    origins = nc.dram_tensor("origins", [N_LOC, 2], F32, kind="ExternalInput").ap()
    directions = nc.dram_tensor("directions", [N_LOC, 2], F32, kind="ExternalInput").ap()
    latents = nc.dram_tensor("latents", [M, 6], F32, kind="ExternalInput").ap()
    prob = nc.dram_tensor("prob", [N_LOC, 1], F32, kind="ExternalOutput").ap()
    build_kernel_body(nc, origins, directions, latents, prob)
    nc.compile()
    return nc


def kernel(origins: np.ndarray, directions: np.ndarray, latents: np.ndarray) -> np.ndarray:
    global _CACHED_NC, LAST_PERF
    assert origins.shape == (N, 2) and directions.shape == (N, 2)
    assert latents.shape == (M, 6)
    origins = np.ascontiguousarray(origins, dtype=np.float32)
    directions = np.ascontiguousarray(directions, dtype=np.float32)
    latents = np.ascontiguousarray(latents, dtype=np.float32)

    if _CACHED_NC is None:
        _CACHED_NC = build_nc()
    nc = _CACHED_NC

    in_maps = []
    for c in range(N_CORES):
        sl = slice(c * N_LOC, (c + 1) * N_LOC)
        in_maps.append(
            {
                "origins": origins[sl],
                "directions": directions[sl],
                "latents": latents,
            }
        )

    results = bass_utils.run_bass_kernel_spmd(
        nc,
        in_maps,
        core_ids=list(range(N_CORES)),
        trace=TRACE,
    )
    LAST_PERF = results
    out = np.concatenate([results.results[c]["prob"] for c in range(N_CORES)], axis=0)
    return out.astype(np.float32)


if __name__ == "__main__":
    rng = np.random.default_rng(0)
    o = rng.standard_normal((N, 2), dtype=np.float32)
    d = rng.standard_normal((N, 2), dtype=np.float32)
    l = rng.standard_normal((M, 6), dtype=np.float32)
    p = kernel(o, d, l)
    print(p.shape, p.dtype, p.min(), p.max())
